# revision 6
# baseline (speedup 1.0000x reference)
"""Trainium2 Bass kernel for a transformer decoder layer (self-attn +
cross-attn + FFN, post-LN), SPMD across 8 NeuronCores.

Sharding: core = (batch b, query-half): each core handles 512 of the 1024
target positions of one batch element, with the full key/value context.
No collectives; the gather is done on host.

On-device layout: activations are kept TRANSPOSED [features(part) x
tokens(free)] so every linear is out^T[f,:] += W[c,f-block]^T @ act^T[c,:]
with the stored [in,out] weight as the stationary operand.  Attention scores
are computed as S^T [keys(part) x queries(free)]; softmax denominators come
from a ones-column matmul; per-query normalization / LayerNorm row
broadcasts use K=1 matmuls.
"""

import os
import sys

for _p in ("/opt/trn_rl_repo", "/root/.axon_site/_ro/trn_rl_repo"):
    if os.path.isdir(_p) and _p not in sys.path:
        sys.path.insert(0, _p)

import numpy as np
import ml_dtypes

D_MODEL = 1024
N_HEADS = 16
DH = 64
D_FF = 4096
B = 4
TGT = 1024
SRC = 2048
EPS = 1e-5

P = 128
NQ = TGT // 2          # queries per core = 512
CT = D_MODEL // P      # 8 feature tiles
KT_SA = TGT // P       # 8 key tiles (self)
KT_CA = SRC // P       # 16 key tiles (cross)
FFT = D_FF // P        # 32 ffn tiles

_BUILD_CACHE = {}


def _patch_tile_drain(tile):
    """walrus (CoreV3) rejects multiple sync-waits on one TPB_CTRL Drain;
    split the Tile tail drain into one drain per semaphore wait."""
    if getattr(tile.TileContext, "_drain_patch_installed", False):
        return

    def _drain_and_barrier(self, tick_clock, wait_clock):
        drain_inst = self.nc.sync.drain()
        wait_clock.add_sem_waits(
            drain_inst.ins, tile.ScopedClock({None: tick_clock.global_clock})
        )
        si = drain_inst.ins.sync_info
        if si is not None and si.on_wait and len(si.on_wait) > 1:
            waits = list(si.on_wait)
            del si.on_wait[1:]
            for w in waits[1:]:
                extra = self.nc.sync.drain()
                if extra.ins.sync_info is None:
                    import concourse.mybir as mybir
                    extra.ins.sync_info = mybir.SyncInfo(on_wait=[], on_update=[])
                extra.ins.sync_info.on_wait.append(w)

        self.nc.all_engine_barrier()
        assert self.sems is not None
        popped = self.nc._tile_sem_poison_stack.pop()
        assert popped is self._sem_poison
        self.nc.clear_and_free_semaphores(list(self.sems.allocated().values()))
        self.nc.all_engine_barrier()

    tile.TileContext._drain_and_barrier = _drain_and_barrier
    tile.TileContext._drain_patch_installed = True


def _split_multi_waits(nc, mybir):
    """This walrus build accepts a single sync-wait per instruction; hoist
    extra waits onto standalone EventSemaphore carriers inserted just before
    the instruction on the same engine."""
    n = 0
    for f in nc.m.functions:
        for blk in f.blocks:
            out = []
            for ins in blk.instructions:
                si = getattr(ins, "sync_info", None)
                if si is not None and si.on_wait and len(si.on_wait) > 1:
                    waits = list(si.on_wait)
                    del si.on_wait[:-1]
                    for w in waits[:-1]:
                        n += 1
                        carrier = mybir.InstEventSemaphore(
                            name=f"I-waitsplit-{n}",
                            engine=ins.engine,
                            ins=[],
                            outs=[],
                            sync_info=mybir.SyncInfo(on_wait=[w], on_update=[]),
                        )
                        out.append(carrier)
                out.append(ins)
            blk.instructions = out
    return n


def _build(with_mask_sa, with_mask_ca, with_ln_affine):
    import concourse.bass as bass
    import concourse.mybir as mybir
    import concourse.tile as tile

    _patch_tile_drain(tile)

    f32 = mybir.dt.float32
    bf16 = mybir.dt.bfloat16
    AF = mybir.ActivationFunctionType
    OP = mybir.AluOpType

    nc = bass.Bass("TRN2", target_bir_lowering=False, debug=False, num_devices=1)

    # ---- DRAM I/O (column-block layout: [128, n_tiles*width]) ----
    d_xq = nc.dram_tensor("xq", [P, CT * NQ], bf16, kind="ExternalInput").ap()
    d_xq32 = nc.dram_tensor("xq32", [P, CT * NQ], f32, kind="ExternalInput").ap()
    d_xt = nc.dram_tensor("xt", [P, CT * TGT], bf16, kind="ExternalInput").ap()
    d_enct = nc.dram_tensor("enct", [P, CT * SRC], bf16, kind="ExternalInput").ap()
    dw = {}
    for name in ("sa_wq", "sa_wk", "sa_wv", "sa_wo", "ca_wq", "ca_wk", "ca_wv", "ca_wo"):
        dw[name] = nc.dram_tensor(name, [P, CT * D_MODEL], bf16, kind="ExternalInput").ap()
    dw["ffn_w1"] = nc.dram_tensor("ffn_w1", [P, CT * D_FF], bf16, kind="ExternalInput").ap()
    dw["ffn_w2"] = nc.dram_tensor("ffn_w2", [P, FFT * D_MODEL], bf16, kind="ExternalInput").ap()
    d_fb = nc.dram_tensor("fbias", [P, FFT + CT + 1], f32, kind="ExternalInput").ap()
    d_ones32 = nc.dram_tensor("ones32", [P, P], f32, kind="ExternalInput").ap()
    d_onesbf = nc.dram_tensor("onesbf", [P, 8], bf16, kind="ExternalInput").ap()
    d_mask_sa = d_mask_ca = None
    if with_mask_sa:
        d_mask_sa = nc.dram_tensor("mask_sa", [P, KT_SA * NQ], f32, kind="ExternalInput").ap()
    if with_mask_ca:
        d_mask_ca = nc.dram_tensor("mask_ca", [P, KT_CA * NQ], f32, kind="ExternalInput").ap()
    d_lnp = None
    if with_ln_affine:
        d_lnp = nc.dram_tensor("lnp", [P, 6 * CT], f32, kind="ExternalInput").ap()
    d_out = nc.dram_tensor("out", [P, CT * NQ], f32, kind="ExternalOutput").ap()

    with tile.TileContext(nc) as tc:
        with (
            tc.tile_pool(name="const", bufs=1) as cpool,
            tc.tile_pool(name="ps_proj", bufs=2, space="PSUM") as ps_proj,
            tc.tile_pool(name="ps_sc", bufs=4, space="PSUM") as ps_sc,
            tc.tile_pool(name="ps_ctx", bufs=1, space="PSUM") as ps_ctx,
            tc.tile_pool(name="ps_aux", bufs=1, space="PSUM") as ps_aux,
        ):
            ones32 = cpool.tile([P, P], f32, tag="ones32")
            nc.sync.dma_start(ones32[:], d_ones32[:])
            onesbf = cpool.tile([P, 8], bf16, tag="onesbf")
            nc.sync.dma_start(onesbf[:], d_onesbf[:])
            fbias = cpool.tile([P, FFT + CT + 1], f32, tag="fbias")
            nc.sync.dma_start(fbias[:], d_fb[:])
            lnp = None
            if with_ln_affine:
                lnp = cpool.tile([P, 6 * CT], f32, tag="lnp")
                nc.sync.dma_start(lnp[:], d_lnp[:])

            def layer_norm(res_sb, out32_sb, outbf_sb, tmp_pool, ln_idx):
                """res_sb [128, CT*NQ] f32 -> out32 (f32) and optional outbf
                (bf16); LayerNorm over the feature (partition x tile) axis."""
                mean_ps = ps_ctx.tile([P, NQ], f32, tag="ctx")
                sq_ps = ps_aux.tile([P, NQ], f32, tag="aux")
                for c in range(CT):
                    rsl = res_sb[:, c * NQ:(c + 1) * NQ]
                    sq = tmp_pool.tile([P, NQ], f32, tag="sq", bufs=2)
                    nc.scalar.activation(sq[:], rsl, AF.Square)
                    nc.tensor.matmul(mean_ps[0:1, :], ones32[:, 0:1], rsl,
                                     start=(c == 0), stop=(c == CT - 1))
                    nc.tensor.matmul(sq_ps[0:1, :], ones32[:, 0:1], sq[:],
                                     start=(c == 0), stop=(c == CT - 1))
                rows = tmp_pool.tile([1, 4 * NQ], f32, tag="rows")
                m = rows[:, 0:NQ]
                s = rows[:, NQ:2 * NQ]
                a = rows[:, 2 * NQ:3 * NQ]
                b = rows[:, 3 * NQ:4 * NQ]
                nc.vector.tensor_scalar_mul(m, mean_ps[0:1, :], 1.0 / D_MODEL)
                nc.vector.tensor_scalar_mul(s, sq_ps[0:1, :], 1.0 / D_MODEL)
                nc.vector.tensor_tensor(a, m, m, op=OP.mult)        # mean^2
                nc.vector.tensor_tensor(a, s, a, op=OP.subtract)    # var
                nc.scalar.activation(a, a, AF.Sqrt, bias=fbias[0:1, FFT + CT:FFT + CT + 1])  # sqrt(var+eps)
                nc.vector.reciprocal(a, a)                          # rstd
                nc.vector.tensor_tensor(b, m, a, op=OP.mult)
                nc.vector.tensor_scalar_mul(b, b, -1.0)             # -mean*rstd
                bca_ps = ps_aux.tile([P, NQ], f32, tag="aux")
                bcb_ps = ps_ctx.tile([P, NQ], f32, tag="ctx")
                nc.tensor.matmul(bca_ps[:, :], ones32[0:1, :], a, start=True, stop=True)
                nc.tensor.matmul(bcb_ps[:, :], ones32[0:1, :], b, start=True, stop=True)
                bca = tmp_pool.tile([P, NQ], f32, tag="bca")
                bcb = tmp_pool.tile([P, NQ], f32, tag="bcb")
                nc.scalar.copy(bca[:], bca_ps[:, :])
                nc.scalar.copy(bcb[:], bcb_ps[:, :])
                for c in range(CT):
                    rsl = res_sb[:, c * NQ:(c + 1) * NQ]
                    o32 = out32_sb[:, c * NQ:(c + 1) * NQ]
                    t = tmp_pool.tile([P, NQ], f32, tag="sq", bufs=2)
                    nc.vector.tensor_tensor(t[:], rsl, bca[:], op=OP.mult)
                    nc.vector.tensor_tensor(o32, t[:], bcb[:], op=OP.add)
                    if with_ln_affine:
                        g = lnp[:, (2 * ln_idx) * CT + c:(2 * ln_idx) * CT + c + 1]
                        be = lnp[:, (2 * ln_idx + 1) * CT + c:(2 * ln_idx + 1) * CT + c + 1]
                        nc.vector.tensor_scalar(o32, o32, g, be, op0=OP.mult, op1=OP.add)
                    if outbf_sb is not None:
                        nc.scalar.copy(outbf_sb[:, c * NQ:(c + 1) * NQ], o32)

            def attention(qt, kt, v, nkt, ctxn, probs_pool, rec_pool, mask_sb):
                """qt [128, CT*NQ] bf16, kt [128, CT*(nkt*128)] bf16,
                v [128, nkt*D_MODEL] bf16 -> ctxn [128, CT*NQ] bf16."""
                KW = nkt * P  # key width per feature block of kt
                for pair in range(N_HEADS // 2):
                    t = pair
                    ctx_ps = ps_ctx.tile([P, NQ], f32, tag="ctx")
                    den_ps = ps_aux.tile([P, NQ], f32, tag="aux")
                    for k in range(nkt):
                        for half in range(2):
                            h = 2 * pair + half
                            psl = slice(64 * half, 64 * half + 64)
                            s_ps = ps_sc.tile([P, NQ], f32, tag="sc")
                            nc.tensor.matmul(
                                s_ps[:, :],
                                kt[psl, t * KW + k * P: t * KW + (k + 1) * P],
                                qt[psl, t * NQ:(t + 1) * NQ],
                                start=True, stop=True)
                            if mask_sb is not None:
                                nc.vector.tensor_tensor(
                                    s_ps[:, :], s_ps[:, :],
                                    mask_sb[:, k * NQ:(k + 1) * NQ], op=OP.add)
                            p_sb = probs_pool.tile([P, NQ], bf16, tag="p")
                            nc.scalar.activation(p_sb[:], s_ps[:, :], AF.Exp,
                                                 scale=1.0 / 8.0)
                            nc.tensor.matmul(
                                ctx_ps[psl, :],
                                v[:, k * D_MODEL + h * DH: k * D_MODEL + (h + 1) * DH],
                                p_sb[:], start=(k == 0), stop=(k == nkt - 1))
                            nc.tensor.matmul(
                                den_ps[64 * half:64 * half + 1, :],
                                onesbf[:, 0:1],
                                p_sb[:], start=(k == 0), stop=(k == nkt - 1))
                    rec = rec_pool.tile([P, NQ], f32, tag="rec")
                    nc.vector.reciprocal(rec[0:1, :], den_ps[0:1, :])
                    nc.vector.reciprocal(rec[64:65, :], den_ps[64:65, :])
                    bc_ps = ps_sc.tile([P, NQ], f32, tag="sc")
                    nc.tensor.matmul(bc_ps[0:64, :], ones32[0:1, 0:64], rec[0:1, :],
                                     start=True, stop=True)
                    nc.tensor.matmul(bc_ps[64:128, :], ones32[64:65, 0:64], rec[64:65, :],
                                     start=True, stop=True)
                    bc = rec_pool.tile([P, NQ], f32, tag="bc")
                    nc.scalar.copy(bc[:], bc_ps[:, :])
                    nc.vector.tensor_tensor(ctxn[:, t * NQ:(t + 1) * NQ],
                                            ctx_ps[:, :], bc[:], op=OP.mult)

            def proj_T(w_sb, act_sb, actw, dst_sb, dstw, nchunks):
                """dst^T[f-block, chunk] = sum_c W[c, f]^T @ act^T[c, chunk]"""
                for f in range(CT):
                    for n in range(nchunks):
                        ps = ps_proj.tile([P, NQ], f32, tag="proj")
                        for c in range(CT):
                            nc.tensor.matmul(
                                ps[:, :],
                                w_sb[:, c * D_MODEL + f * P: c * D_MODEL + (f + 1) * P],
                                act_sb[:, c * actw + n * NQ: c * actw + (n + 1) * NQ],
                                start=(c == 0), stop=(c == CT - 1))
                        nc.vector.tensor_copy(
                            dst_sb[:, f * dstw + n * NQ: f * dstw + (n + 1) * NQ],
                            ps[:, :])

            def v_proj(act_sb, actw, wv_sb, v_sb, nkt):
                """V[k-block, :] = act[k,:] @ Wv  (normal layout)."""
                for k in range(nkt):
                    for n in range(D_MODEL // NQ):
                        ps = ps_proj.tile([P, NQ], f32, tag="proj")
                        for c in range(CT):
                            nc.tensor.matmul(
                                ps[:, :],
                                act_sb[:, c * actw + k * P: c * actw + (k + 1) * P],
                                wv_sb[:, c * D_MODEL + n * NQ: c * D_MODEL + (n + 1) * NQ],
                                start=(c == 0), stop=(c == CT - 1))
                        nc.vector.tensor_copy(
                            v_sb[:, k * D_MODEL + n * NQ: k * D_MODEL + (n + 1) * NQ],
                            ps[:, :])

            with tc.tile_pool(name="s2out", bufs=1) as s2out:
                o2_32 = s2out.tile([P, CT * NQ], f32, tag="o2_32")
                o2_bf = s2out.tile([P, CT * NQ], bf16, tag="o2_bf")

                with (
                    tc.tile_pool(name="s1out", bufs=1) as s1out,
                    tc.tile_pool(name="ctxn_pool", bufs=1) as ctxn_pool,
                    tc.tile_pool(name="probs", bufs=6) as probs_pool,
                    tc.tile_pool(name="rec", bufs=2) as rec_pool,
                ):
                    o1_32 = s1out.tile([P, CT * NQ], f32, tag="o1_32")
                    o1_bf = s1out.tile([P, CT * NQ], bf16, tag="o1_bf")
                    ctxn = ctxn_pool.tile([P, CT * NQ], bf16, tag="ctxn")

                    # ---------------- self attention ----------------
                    with tc.tile_pool(name="saP", bufs=1) as saP:
                        xq = saP.tile([P, CT * NQ], bf16, tag="xq")
                        nc.sync.dma_start(xq[:], d_xq[:])
                        xt = saP.tile([P, CT * TGT], bf16, tag="xt")
                        nc.sync.dma_start(xt[:], d_xt[:])
                        wq = saP.tile([P, CT * D_MODEL], bf16, tag="wq")
                        nc.sync.dma_start(wq[:], dw["sa_wq"][:])
                        wk = saP.tile([P, CT * D_MODEL], bf16, tag="wk")
                        nc.sync.dma_start(wk[:], dw["sa_wk"][:])
                        wv = saP.tile([P, CT * D_MODEL], bf16, tag="wv")
                        nc.sync.dma_start(wv[:], dw["sa_wv"][:])
                        mask_sa = None
                        if with_mask_sa:
                            mask_sa = saP.tile([P, KT_SA * NQ], f32, tag="msa")
                            nc.sync.dma_start(mask_sa[:], d_mask_sa[:])

                        qt = saP.tile([P, CT * NQ], bf16, tag="qt")
                        kt = saP.tile([P, CT * TGT], bf16, tag="kt")
                        v = saP.tile([P, KT_SA * D_MODEL], bf16, tag="v")

                        proj_T(wq, xq, NQ, qt, NQ, 1)
                        proj_T(wk, xt, TGT, kt, TGT, TGT // NQ)
                        v_proj(xt, TGT, wv, v, KT_SA)
                        attention(qt, kt, v, KT_SA, ctxn, probs_pool, rec_pool, mask_sa)

                    with tc.tile_pool(name="saQ", bufs=1) as saQ:
                        wo = saQ.tile([P, CT * D_MODEL], bf16, tag="wo")
                        nc.sync.dma_start(wo[:], dw["sa_wo"][:])
                        xq32 = saQ.tile([P, CT * NQ], f32, tag="xq32")
                        nc.sync.dma_start(xq32[:], d_xq32[:])
                        res1 = saQ.tile([P, CT * NQ], f32, tag="res1")
                        for f in range(CT):
                            ps = ps_proj.tile([P, NQ], f32, tag="proj")
                            for c in range(CT):
                                nc.tensor.matmul(
                                    ps[:, :],
                                    wo[:, c * D_MODEL + f * P: c * D_MODEL + (f + 1) * P],
                                    ctxn[:, c * NQ:(c + 1) * NQ],
                                    start=(c == 0), stop=(c == CT - 1))
                            nc.vector.tensor_tensor(
                                res1[:, f * NQ:(f + 1) * NQ], ps[:, :],
                                xq32[:, f * NQ:(f + 1) * NQ], op=OP.add)
                        layer_norm(res1, o1_32, o1_bf, saQ, 0)

                    # ---------------- cross attention ----------------
                    with tc.tile_pool(name="caP", bufs=1) as caP:
                        qt = caP.tile([P, CT * NQ], bf16, tag="qt")
                        kt = caP.tile([P, CT * SRC], bf16, tag="kt")
                        v = caP.tile([P, KT_CA * D_MODEL], bf16, tag="v")
                        mask_ca = None
                        if with_mask_ca:
                            mask_ca = caP.tile([P, KT_CA * NQ], f32, tag="mca")
                            nc.sync.dma_start(mask_ca[:], d_mask_ca[:])

                        with tc.tile_pool(name="caP1a", bufs=1) as caP1a:
                            enct = caP1a.tile([P, CT * SRC], bf16, tag="enct")
                            nc.sync.dma_start(enct[:], d_enct[:])
                            wk = caP1a.tile([P, CT * D_MODEL], bf16, tag="wk")
                            nc.sync.dma_start(wk[:], dw["ca_wk"][:])
                            wv = caP1a.tile([P, CT * D_MODEL], bf16, tag="wv")
                            nc.sync.dma_start(wv[:], dw["ca_wv"][:])
                            proj_T(wk, enct, SRC, kt, SRC, SRC // NQ)
                            v_proj(enct, SRC, wv, v, KT_CA)

                        with tc.tile_pool(name="caP1b", bufs=1) as caP1b:
                            wq = caP1b.tile([P, CT * D_MODEL], bf16, tag="wq")
                            nc.sync.dma_start(wq[:], dw["ca_wq"][:])
                            proj_T(wq, o1_bf, NQ, qt, NQ, 1)

                        attention(qt, kt, v, KT_CA, ctxn, probs_pool, rec_pool, mask_ca)

                    with tc.tile_pool(name="caQ", bufs=1) as caQ:
                        wo = caQ.tile([P, CT * D_MODEL], bf16, tag="wo")
                        nc.sync.dma_start(wo[:], dw["ca_wo"][:])
                        res2 = caQ.tile([P, CT * NQ], f32, tag="res2")
                        for f in range(CT):
                            ps = ps_proj.tile([P, NQ], f32, tag="proj")
                            for c in range(CT):
                                nc.tensor.matmul(
                                    ps[:, :],
                                    wo[:, c * D_MODEL + f * P: c * D_MODEL + (f + 1) * P],
                                    ctxn[:, c * NQ:(c + 1) * NQ],
                                    start=(c == 0), stop=(c == CT - 1))
                            nc.vector.tensor_tensor(
                                res2[:, f * NQ:(f + 1) * NQ], ps[:, :],
                                o1_32[:, f * NQ:(f + 1) * NQ], op=OP.add)
                        layer_norm(res2, o2_32, o2_bf, caQ, 1)

                # ---------------- FFN ----------------
                with tc.tile_pool(name="hid_pool", bufs=1) as hid_pool:
                    hid = hid_pool.tile([P, FFT * NQ], bf16, tag="hid")
                    with tc.tile_pool(name="f1", bufs=1) as f1:
                        w1 = f1.tile([P, CT * D_FF], bf16, tag="w1")
                        nc.sync.dma_start(w1[:], dw["ffn_w1"][:])
                        for ff in range(FFT):
                            ps = ps_proj.tile([P, NQ], f32, tag="proj")
                            for c in range(CT):
                                nc.tensor.matmul(
                                    ps[:, :],
                                    w1[:, c * D_FF + ff * P: c * D_FF + (ff + 1) * P],
                                    o2_bf[:, c * NQ:(c + 1) * NQ],
                                    start=(c == 0), stop=(c == CT - 1))
                            nc.scalar.activation(
                                hid[:, ff * NQ:(ff + 1) * NQ], ps[:, :], AF.Relu,
                                bias=fbias[:, ff:ff + 1])
                    with tc.tile_pool(name="f2", bufs=1) as f2:
                        w2 = f2.tile([P, FFT * D_MODEL], bf16, tag="w2")
                        nc.sync.dma_start(w2[:], dw["ffn_w2"][:])
                        res3 = f2.tile([P, CT * NQ], f32, tag="res3")
                        o3_32 = f2.tile([P, CT * NQ], f32, tag="o3_32")
                        for f in range(CT):
                            ps = ps_proj.tile([P, NQ], f32, tag="proj")
                            for c in range(FFT):
                                nc.tensor.matmul(
                                    ps[:, :],
                                    w2[:, c * D_MODEL + f * P: c * D_MODEL + (f + 1) * P],
                                    hid[:, c * NQ:(c + 1) * NQ],
                                    start=(c == 0), stop=(c == FFT - 1))
                            nc.vector.scalar_tensor_tensor(
                                res3[:, f * NQ:(f + 1) * NQ], ps[:, :],
                                fbias[:, FFT + f:FFT + f + 1],
                                o2_32[:, f * NQ:(f + 1) * NQ],
                                op0=OP.add, op1=OP.add)
                        layer_norm(res3, o3_32, None, f2, 2)
                        nc.sync.dma_start(d_out[:], o3_32[:])

    _split_multi_waits(nc, mybir)
    return nc


def _ln_is_trivial(g, b):
    return bool(np.all(np.asarray(g) == 1.0) and np.all(np.asarray(b) == 0.0))


def _to_blocks(a, width):
    """[n_tiles*128, width] -> [128, n_tiles*width] column-block layout."""
    n = a.shape[0] // P
    return np.ascontiguousarray(
        a.reshape(n, P, width).transpose(1, 0, 2).reshape(P, n * width))


def kernel(**inputs):
    from concourse import bass_utils

    x = np.asarray(inputs["dec_layer_inputs"], np.float32)       # [B, TGT, DM]
    enc = np.asarray(inputs["enc_outputs"], np.float32)          # [B, SRC, DM]
    m_sa = np.asarray(inputs["dec_self_attn_mask"], np.float32)  # [B,1,TGT,TGT]
    m_ca = np.asarray(inputs["dec_enc_attn_mask"], np.float32)   # [B,1,TGT,SRC]

    with_mask_sa = bool(np.any(m_sa))
    with_mask_ca = bool(np.any(m_ca))
    with_ln_affine = not (
        _ln_is_trivial(inputs["ln1_g"], inputs["ln1_b"])
        and _ln_is_trivial(inputs["ln2_g"], inputs["ln2_b"])
        and _ln_is_trivial(inputs["ln3_g"], inputs["ln3_b"]))

    key = (with_mask_sa, with_mask_ca, with_ln_affine)
    if key not in _BUILD_CACHE:
        _BUILD_CACHE[key] = _build(*key)
    nc = _BUILD_CACHE[key]

    bf = ml_dtypes.bfloat16

    def wblocks(name, width):
        return _to_blocks(np.asarray(inputs[name], np.float32), width).astype(bf)

    shared = {
        "sa_wq": wblocks("sa_wq", D_MODEL), "sa_wk": wblocks("sa_wk", D_MODEL),
        "sa_wv": wblocks("sa_wv", D_MODEL), "sa_wo": wblocks("sa_wo", D_MODEL),
        "ca_wq": wblocks("ca_wq", D_MODEL), "ca_wk": wblocks("ca_wk", D_MODEL),
        "ca_wv": wblocks("ca_wv", D_MODEL), "ca_wo": wblocks("ca_wo", D_MODEL),
        "ffn_w1": wblocks("ffn_w1", D_FF),
        "ffn_w2": wblocks("ffn_w2", D_MODEL),
        "ones32": np.ones((P, P), np.float32),
        "onesbf": np.ones((P, 8), bf),
    }
    fb = np.zeros((P, FFT + CT + 1), np.float32)
    fb[:, FFT + CT] = EPS
    fb[:, :FFT] = np.asarray(inputs["ffn_b1"], np.float32).reshape(FFT, P).T
    fb[:, FFT:FFT + CT] = np.asarray(inputs["ffn_b2"], np.float32).reshape(CT, P).T
    shared["fbias"] = fb
    if with_ln_affine:
        lp = np.zeros((P, 6 * CT), np.float32)
        for i, nm in enumerate(("ln1_g", "ln1_b", "ln2_g", "ln2_b", "ln3_g", "ln3_b")):
            lp[:, i * CT:(i + 1) * CT] = np.asarray(inputs[nm], np.float32).reshape(CT, P).T
        shared["lnp"] = lp

    in_maps = []
    for core in range(8):
        b, half = divmod(core, 2)
        q0 = half * NQ
        xT = x[b].T                      # [DM, TGT]
        encT = enc[b].T                  # [DM, SRC]
        im = dict(shared)
        im["xq"] = _to_blocks(np.ascontiguousarray(xT[:, q0:q0 + NQ]), NQ).astype(bf)
        im["xq32"] = _to_blocks(np.ascontiguousarray(xT[:, q0:q0 + NQ]), NQ)
        im["xt"] = _to_blocks(np.ascontiguousarray(xT), TGT).astype(bf)
        im["enct"] = _to_blocks(np.ascontiguousarray(encT), SRC).astype(bf)
        if with_mask_sa:
            im["mask_sa"] = _to_blocks(np.ascontiguousarray(m_sa[b, 0].T[:, q0:q0 + NQ]), NQ)
        if with_mask_ca:
            im["mask_ca"] = _to_blocks(np.ascontiguousarray(m_ca[b, 0].T[:, q0:q0 + NQ]), NQ)
        in_maps.append(im)

    trace = bool(int(os.environ.get("KERNEL_TRACE", "0")))
    res = bass_utils.run_bass_kernel_spmd(
        nc, in_maps, core_ids=list(range(8)), trace=trace)
    kernel.last_results = res

    out = np.empty((B, TGT, D_MODEL), np.float32)
    for core in range(8):
        b, half = divmod(core, 2)
        q0 = half * NQ
        o = np.asarray(res.results[core]["out"])   # [128, CT*NQ]
        oT = o.reshape(P, CT, NQ).transpose(1, 0, 2).reshape(D_MODEL, NQ)
        out[b, q0:q0 + NQ, :] = oT.T
    return out


# revision 12
# speedup vs baseline: 1.2725x; 1.2725x over previous
"""Trainium2 Bass kernel for a transformer decoder layer (self-attn +
cross-attn + FFN, post-LN), SPMD across 8 NeuronCores.

Sharding: core = (batch b, query-half): each core handles 512 of the 1024
target positions of one batch element, with the full key/value context.
No collectives; the gather is done on host.

On-device layout: activations are kept TRANSPOSED [features(part) x
tokens(free)] so every linear is out^T[f,:] += W[c,f-block]^T @ act^T[c,:]
with the stored [in,out] weight as the stationary operand.  Attention scores
are computed as S^T [keys(part) x queries(free)]; softmax denominators come
from a ones-column matmul; per-query normalization / LayerNorm row
broadcasts use K=1 matmuls.
"""

import os
import sys

for _p in ("/opt/trn_rl_repo", "/root/.axon_site/_ro/trn_rl_repo"):
    if os.path.isdir(_p) and _p not in sys.path:
        sys.path.insert(0, _p)

import numpy as np
import ml_dtypes

D_MODEL = 1024
N_HEADS = 16
DH = 64
D_FF = 4096
B = 4
TGT = 1024
SRC = 2048
EPS = 1e-5

P = 128
NQ = TGT // 2          # queries per core = 512
CT = D_MODEL // P      # 8 feature tiles
KT_SA = TGT // P       # 8 key tiles (self)
KT_CA = SRC // P       # 16 key tiles (cross)
FFT = D_FF // P        # 32 ffn tiles

_BUILD_CACHE = {}


def _patch_tile_drain(tile):
    """walrus (CoreV3) rejects multiple sync-waits on one TPB_CTRL Drain;
    split the Tile tail drain into one drain per semaphore wait."""
    if getattr(tile.TileContext, "_drain_patch_installed", False):
        return

    def _drain_and_barrier(self, tick_clock, wait_clock):
        drain_inst = self.nc.sync.drain()
        wait_clock.add_sem_waits(
            drain_inst.ins, tile.ScopedClock({None: tick_clock.global_clock})
        )
        si = drain_inst.ins.sync_info
        if si is not None and si.on_wait and len(si.on_wait) > 1:
            waits = list(si.on_wait)
            del si.on_wait[1:]
            for w in waits[1:]:
                extra = self.nc.sync.drain()
                if extra.ins.sync_info is None:
                    import concourse.mybir as mybir
                    extra.ins.sync_info = mybir.SyncInfo(on_wait=[], on_update=[])
                extra.ins.sync_info.on_wait.append(w)

        self.nc.all_engine_barrier()
        assert self.sems is not None
        popped = self.nc._tile_sem_poison_stack.pop()
        assert popped is self._sem_poison
        self.nc.clear_and_free_semaphores(list(self.sems.allocated().values()))
        self.nc.all_engine_barrier()

    tile.TileContext._drain_and_barrier = _drain_and_barrier
    tile.TileContext._drain_patch_installed = True


def _split_multi_waits(nc, mybir):
    """This walrus build accepts a single sync-wait per instruction; hoist
    extra waits onto standalone EventSemaphore carriers inserted just before
    the instruction on the same engine."""
    n = 0
    for f in nc.m.functions:
        for blk in f.blocks:
            out = []
            for ins in blk.instructions:
                si = getattr(ins, "sync_info", None)
                if si is not None and si.on_wait and len(si.on_wait) > 1:
                    waits = list(si.on_wait)
                    del si.on_wait[:-1]
                    for w in waits[:-1]:
                        n += 1
                        carrier = mybir.InstEventSemaphore(
                            name=f"I-waitsplit-{n}",
                            engine=ins.engine,
                            ins=[],
                            outs=[],
                            sync_info=mybir.SyncInfo(on_wait=[w], on_update=[]),
                        )
                        out.append(carrier)
                out.append(ins)
            blk.instructions = out
    return n


def _build(with_mask_sa, with_mask_ca, with_ln_affine):
    import concourse.bass as bass
    import concourse.mybir as mybir
    import concourse.tile as tile

    _patch_tile_drain(tile)

    f32 = mybir.dt.float32
    bf16 = mybir.dt.bfloat16
    AF = mybir.ActivationFunctionType
    OP = mybir.AluOpType

    nc = bass.Bass("TRN2", target_bir_lowering=False, debug=False, num_devices=1)

    # ---- DRAM I/O (column-block layout: [128, n_tiles*width]) ----
    d_xq = nc.dram_tensor("xq", [P, CT * NQ], bf16, kind="ExternalInput").ap()
    d_xq32 = nc.dram_tensor("xq32", [P, CT * NQ], f32, kind="ExternalInput").ap()
    d_xt = nc.dram_tensor("xt", [P, CT * TGT], bf16, kind="ExternalInput").ap()
    d_enct = nc.dram_tensor("enct", [P, CT * SRC], bf16, kind="ExternalInput").ap()
    dw = {}
    for name in ("sa_wq", "sa_wk", "sa_wv", "sa_wo", "ca_wq", "ca_wk", "ca_wv", "ca_wo"):
        dw[name] = nc.dram_tensor(name, [P, CT * D_MODEL], bf16, kind="ExternalInput").ap()
    dw["ffn_w1"] = nc.dram_tensor("ffn_w1", [P, CT * D_FF], bf16, kind="ExternalInput").ap()
    dw["ffn_w2"] = nc.dram_tensor("ffn_w2", [P, FFT * D_MODEL], bf16, kind="ExternalInput").ap()
    d_fb = nc.dram_tensor("fbias", [P, FFT + CT + 1], f32, kind="ExternalInput").ap()
    d_ones32 = nc.dram_tensor("ones32", [P, P], f32, kind="ExternalInput").ap()
    d_onesbf = nc.dram_tensor("onesbf", [P, 8], bf16, kind="ExternalInput").ap()
    d_mask_sa = d_mask_ca = None
    if with_mask_sa:
        d_mask_sa = nc.dram_tensor("mask_sa", [P, KT_SA * NQ], f32, kind="ExternalInput").ap()
    if with_mask_ca:
        d_mask_ca = nc.dram_tensor("mask_ca", [P, KT_CA * NQ], f32, kind="ExternalInput").ap()
    d_lnp = None
    if with_ln_affine:
        d_lnp = nc.dram_tensor("lnp", [P, 6 * CT], f32, kind="ExternalInput").ap()
    d_out = nc.dram_tensor("out", [P, CT * NQ], f32, kind="ExternalOutput").ap()

    with tile.TileContext(nc) as tc:
        with (
            tc.tile_pool(name="const", bufs=1) as cpool,
            tc.tile_pool(name="ps_proj", bufs=2, space="PSUM") as ps_proj,
            tc.tile_pool(name="ps_sc", bufs=2, space="PSUM") as ps_sc,
            tc.tile_pool(name="ps_ctx", bufs=1, space="PSUM") as ps_ctx,
            tc.tile_pool(name="ps_aux", bufs=1, space="PSUM") as ps_aux,
        ):
            ones32 = cpool.tile([P, P], f32, tag="ones32")
            nc.sync.dma_start(ones32[:], d_ones32[:])
            onesbf = cpool.tile([P, 8], bf16, tag="onesbf")
            nc.sync.dma_start(onesbf[:], d_onesbf[:])
            fbias = cpool.tile([P, FFT + CT + 1], f32, tag="fbias")
            nc.sync.dma_start(fbias[:], d_fb[:])
            lnp = None
            if with_ln_affine:
                lnp = cpool.tile([P, 6 * CT], f32, tag="lnp")
                nc.sync.dma_start(lnp[:], d_lnp[:])

            def layer_norm(res_sb, out32_sb, outbf_sb, tmp_pool, ln_idx):
                """res_sb [128, CT*NQ] f32 -> out32 (f32) and optional outbf
                (bf16); LayerNorm over the feature (partition x tile) axis."""
                mean_ps = ps_ctx.tile([P, NQ], f32, tag="ctx")
                sq_ps = ps_aux.tile([P, NQ], f32, tag="aux")
                for c in range(CT):
                    rsl = res_sb[:, c * NQ:(c + 1) * NQ]
                    sq = tmp_pool.tile([P, NQ], f32, tag="sq", bufs=2)
                    nc.scalar.activation(sq[:], rsl, AF.Square)
                    nc.tensor.matmul(mean_ps[0:1, :], ones32[:, 0:1], rsl,
                                     start=(c == 0), stop=(c == CT - 1))
                    nc.tensor.matmul(sq_ps[0:1, :], ones32[:, 0:1], sq[:],
                                     start=(c == 0), stop=(c == CT - 1))
                rows = tmp_pool.tile([1, 4 * NQ], f32, tag="rows")
                m = rows[:, 0:NQ]
                s = rows[:, NQ:2 * NQ]
                a = rows[:, 2 * NQ:3 * NQ]
                b = rows[:, 3 * NQ:4 * NQ]
                nc.vector.tensor_scalar_mul(m, mean_ps[0:1, :], 1.0 / D_MODEL)
                nc.vector.tensor_scalar_mul(s, sq_ps[0:1, :], 1.0 / D_MODEL)
                nc.vector.tensor_tensor(a, m, m, op=OP.mult)        # mean^2
                nc.vector.tensor_tensor(a, s, a, op=OP.subtract)    # var
                nc.scalar.activation(a, a, AF.Sqrt, bias=fbias[0:1, FFT + CT:FFT + CT + 1])  # sqrt(var+eps)
                nc.vector.reciprocal(a, a)                          # rstd
                nc.vector.tensor_tensor(b, m, a, op=OP.mult)
                nc.vector.tensor_scalar_mul(b, b, -1.0)             # -mean*rstd
                bca_ps = ps_aux.tile([P, NQ], f32, tag="aux")
                bcb_ps = ps_ctx.tile([P, NQ], f32, tag="ctx")
                nc.tensor.matmul(bca_ps[:, :], ones32[0:1, :], a, start=True, stop=True)
                nc.tensor.matmul(bcb_ps[:, :], ones32[0:1, :], b, start=True, stop=True)
                bca = tmp_pool.tile([P, NQ], f32, tag="bca")
                bcb = tmp_pool.tile([P, NQ], f32, tag="bcb")
                nc.scalar.copy(bca[:], bca_ps[:, :])
                nc.scalar.copy(bcb[:], bcb_ps[:, :])
                for c in range(CT):
                    rsl = res_sb[:, c * NQ:(c + 1) * NQ]
                    o32 = out32_sb[:, c * NQ:(c + 1) * NQ]
                    t = tmp_pool.tile([P, NQ], f32, tag="sq", bufs=2)
                    nc.vector.tensor_tensor(t[:], rsl, bca[:], op=OP.mult)
                    nc.vector.tensor_tensor(o32, t[:], bcb[:], op=OP.add)
                    if with_ln_affine:
                        g = lnp[:, (2 * ln_idx) * CT + c:(2 * ln_idx) * CT + c + 1]
                        be = lnp[:, (2 * ln_idx + 1) * CT + c:(2 * ln_idx + 1) * CT + c + 1]
                        nc.vector.tensor_scalar(o32, o32, g, be, op0=OP.mult, op1=OP.add)
                    if outbf_sb is not None:
                        nc.scalar.copy(outbf_sb[:, c * NQ:(c + 1) * NQ], o32)

            def attention(qt, kt, v, nkt, ctxn, probs_pool, rec_pool, mask_sb,
                          filler=None):
                """qt [128, CT*NQ] bf16, kt [128, CT*(nkt*128)] bf16,
                v [128, nkt*D_MODEL] bf16 -> ctxn [128, CT*NQ] bf16.

                Software-pipelined: scores+exp for step k are issued before the
                ctx/den matmuls of step k-1, so the PE never stalls on the exp.
                `filler` is an optional generator; one next() per pair lets
                independent PE work (e.g. the next phase's projections) fill
                the exp-bound gaps."""
                KW = nkt * P  # key width per feature block of kt

                def emit_cd(k, p2):
                    for half in range(2):
                        h_off = (2 * pair + half) * DH
                        psl = slice(64 * half, 64 * half + 64)
                        pq = p2[:, half * NQ:(half + 1) * NQ]
                        nc.tensor.matmul(
                            ctx_ps[psl, :],
                            v[:, k * D_MODEL + h_off: k * D_MODEL + h_off + DH],
                            pq, start=(k == 0), stop=(k == nkt - 1))
                        nc.tensor.matmul(
                            den_ps[64 * half:64 * half + 1, :],
                            onesbf[:, 0:1],
                            pq, start=(k == 0), stop=(k == nkt - 1))

                for pair in range(N_HEADS // 2):
                    t = pair
                    ctx_ps = ps_ctx.tile([P, NQ], f32, tag="ctx")
                    den_ps = ps_aux.tile([P, NQ], f32, tag="aux")
                    prev = None
                    for k in range(nkt):
                        s2_ps = ps_sc.tile([P, 2 * NQ], f32, tag="sc")
                        for half in range(2):
                            psl = slice(64 * half, 64 * half + 64)
                            nc.tensor.matmul(
                                s2_ps[:, half * NQ:(half + 1) * NQ],
                                kt[psl, t * KW + k * P: t * KW + (k + 1) * P],
                                qt[psl, t * NQ:(t + 1) * NQ],
                                start=True, stop=True)
                        if mask_sb is not None:
                            for half in range(2):
                                nc.vector.tensor_tensor(
                                    s2_ps[:, half * NQ:(half + 1) * NQ],
                                    s2_ps[:, half * NQ:(half + 1) * NQ],
                                    mask_sb[:, k * NQ:(k + 1) * NQ], op=OP.add)
                        p2 = probs_pool.tile([P, 2 * NQ], bf16, tag="p")
                        nc.scalar.activation(p2[:], s2_ps[:, :], AF.Exp,
                                             scale=1.0 / 8.0)
                        if prev is not None:
                            emit_cd(*prev)
                        prev = (k, p2)
                    emit_cd(*prev)
                    # normalize: 1/den broadcast over the pair's 2x64 rows
                    rec = rec_pool.tile([P, NQ], f32, tag="rec")
                    nc.vector.reciprocal(rec[:, :], den_ps[:, :])
                    bc_ps = ps_sc.tile([P, NQ], f32, tag="sc")
                    nc.tensor.matmul(bc_ps[0:64, :], ones32[0:1, 0:64], rec[0:1, :],
                                     start=True, stop=True)
                    nc.tensor.matmul(bc_ps[64:128, :], ones32[64:65, 0:64], rec[64:65, :],
                                     start=True, stop=True)
                    bc = rec_pool.tile([P, NQ], f32, tag="bc")
                    nc.scalar.copy(bc[:], bc_ps[:, :])
                    nc.vector.tensor_tensor(ctxn[:, t * NQ:(t + 1) * NQ],
                                            ctx_ps[:, :], bc[:], op=OP.mult)
                    if filler is not None:
                        next(filler, None)

            def proj_T(w_sb, act_sb, actw, dst_sb, dstw, nchunks):
                """dst^T[f-block, chunk] = sum_c W[c, f]^T @ act^T[c, chunk]"""
                for f in range(CT):
                    for n in range(nchunks):
                        ps = ps_proj.tile([P, NQ], f32, tag="proj")
                        for c in range(CT):
                            nc.tensor.matmul(
                                ps[:, :],
                                w_sb[:, c * D_MODEL + f * P: c * D_MODEL + (f + 1) * P],
                                act_sb[:, c * actw + n * NQ: c * actw + (n + 1) * NQ],
                                start=(c == 0), stop=(c == CT - 1))
                        nc.vector.tensor_copy(
                            dst_sb[:, f * dstw + n * NQ: f * dstw + (n + 1) * NQ],
                            ps[:, :])

            def v_proj(act_sb, actw, wv_sb, v_sb, nkt):
                """V[k-block, :] = act[k,:] @ Wv  (normal layout)."""
                for k in range(nkt):
                    for n in range(D_MODEL // NQ):
                        ps = ps_proj.tile([P, NQ], f32, tag="proj")
                        for c in range(CT):
                            nc.tensor.matmul(
                                ps[:, :],
                                act_sb[:, c * actw + k * P: c * actw + (k + 1) * P],
                                wv_sb[:, c * D_MODEL + n * NQ: c * D_MODEL + (n + 1) * NQ],
                                start=(c == 0), stop=(c == CT - 1))
                        nc.vector.tensor_copy(
                            v_sb[:, k * D_MODEL + n * NQ: k * D_MODEL + (n + 1) * NQ],
                            ps[:, :])

            with tc.tile_pool(name="s2out", bufs=1) as s2out:
                o2_32 = s2out.tile([P, CT * NQ], f32, tag="o2_32")
                o2_bf = s2out.tile([P, CT * NQ], bf16, tag="o2_bf")

                with (
                    tc.tile_pool(name="ctxn_pool", bufs=1) as ctxn_pool,
                    tc.tile_pool(name="probs", bufs=3) as probs_pool,
                    tc.tile_pool(name="rec", bufs=1) as rec_pool,
                    tc.tile_pool(name="ca_ktP", bufs=1) as ca_ktP,
                ):
                    ctxn = ctxn_pool.tile([P, CT * NQ], bf16, tag="ctxn")
                    ca_kt = ca_ktP.tile([P, CT * SRC], bf16, tag="kt")

                    # ---------------- self attention ----------------
                    with tc.tile_pool(name="saAtt", bufs=1) as saAtt:
                        qt = saAtt.tile([P, CT * NQ], bf16, tag="qt")
                        kt = saAtt.tile([P, CT * TGT], bf16, tag="kt")
                        v = saAtt.tile([P, KT_SA * D_MODEL], bf16, tag="v")
                        mask_sa = None
                        if with_mask_sa:
                            mask_sa = saAtt.tile([P, KT_SA * NQ], f32, tag="msa")
                            nc.sync.dma_start(mask_sa[:], d_mask_sa[:])

                        with tc.tile_pool(name="saIn", bufs=1) as saIn:
                            xq = saIn.tile([P, CT * NQ], bf16, tag="xq")
                            nc.sync.dma_start(xq[:], d_xq[:])
                            xt = saIn.tile([P, CT * TGT], bf16, tag="xt")
                            nc.sync.dma_start(xt[:], d_xt[:])
                            wq = saIn.tile([P, CT * D_MODEL], bf16, tag="wq")
                            nc.sync.dma_start(wq[:], dw["sa_wq"][:])
                            wk = saIn.tile([P, CT * D_MODEL], bf16, tag="wk")
                            nc.sync.dma_start(wk[:], dw["sa_wk"][:])
                            wv = saIn.tile([P, CT * D_MODEL], bf16, tag="wv")
                            nc.sync.dma_start(wv[:], dw["sa_wv"][:])
                            proj_T(wq, xq, NQ, qt, NQ, 1)
                            proj_T(wk, xt, TGT, kt, TGT, TGT // NQ)
                            v_proj(xt, TGT, wv, v, KT_SA)

                        with tc.tile_pool(name="caKV", bufs=1) as caKV:
                            enct = caKV.tile([P, CT * SRC], bf16, tag="enct")
                            nc.sync.dma_start(enct[:], d_enct[:])
                            ca_wk = caKV.tile([P, CT * D_MODEL], bf16, tag="wk")
                            nc.sync.dma_start(ca_wk[:], dw["ca_wk"][:])

                            def ca_kt_gen():
                                """cross-attn K^T chains, 4 per next()."""
                                i = 0
                                for f in range(CT):
                                    for n in range(SRC // NQ):
                                        ps = ps_proj.tile([P, NQ], f32, tag="proj")
                                        for c in range(CT):
                                            nc.tensor.matmul(
                                                ps[:, :],
                                                ca_wk[:, c * D_MODEL + f * P: c * D_MODEL + (f + 1) * P],
                                                enct[:, c * SRC + n * NQ: c * SRC + (n + 1) * NQ],
                                                start=(c == 0), stop=(c == CT - 1))
                                        nc.vector.tensor_copy(
                                            ca_kt[:, f * SRC + n * NQ: f * SRC + (n + 1) * NQ],
                                            ps[:, :])
                                        i += 1
                                        if i % 4 == 0:
                                            yield

                            gen = ca_kt_gen()
                            attention(qt, kt, v, KT_SA, ctxn, probs_pool,
                                      rec_pool, mask_sa, filler=gen)
                            for _ in gen:  # finish any remaining chains
                                pass

                    with tc.tile_pool(name="s1out", bufs=1) as s1out:
                        o1_32 = s1out.tile([P, CT * NQ], f32, tag="o1_32")
                        o1_bf = s1out.tile([P, CT * NQ], bf16, tag="o1_bf")

                        with tc.tile_pool(name="saQ", bufs=1) as saQ:
                            wo = saQ.tile([P, CT * D_MODEL], bf16, tag="wo")
                            nc.sync.dma_start(wo[:], dw["sa_wo"][:])
                            xq32 = saQ.tile([P, CT * NQ], f32, tag="xq32")
                            nc.sync.dma_start(xq32[:], d_xq32[:])
                            res1 = saQ.tile([P, CT * NQ], f32, tag="res1")
                            for f in range(CT):
                                ps = ps_proj.tile([P, NQ], f32, tag="proj")
                                for c in range(CT):
                                    nc.tensor.matmul(
                                        ps[:, :],
                                        wo[:, c * D_MODEL + f * P: c * D_MODEL + (f + 1) * P],
                                        ctxn[:, c * NQ:(c + 1) * NQ],
                                        start=(c == 0), stop=(c == CT - 1))
                                nc.vector.tensor_tensor(
                                    res1[:, f * NQ:(f + 1) * NQ], ps[:, :],
                                    xq32[:, f * NQ:(f + 1) * NQ], op=OP.add)
                            layer_norm(res1, o1_32, o1_bf, saQ, 0)

                        # ---------------- cross attention ----------------
                        with tc.tile_pool(name="caVQ", bufs=1) as caVQ:
                            ca_qt = caVQ.tile([P, CT * NQ], bf16, tag="qt")
                            ca_v = caVQ.tile([P, KT_CA * D_MODEL], bf16, tag="v")
                            mask_ca = None
                            if with_mask_ca:
                                mask_ca = caVQ.tile([P, KT_CA * NQ], f32, tag="mca")
                                nc.sync.dma_start(mask_ca[:], d_mask_ca[:])

                            with tc.tile_pool(name="caW2a", bufs=1) as caW2a:
                                ca_wv = caW2a.tile([P, CT * D_MODEL], bf16, tag="wv")
                                nc.sync.dma_start(ca_wv[:], dw["ca_wv"][:])
                                ca_enct = caW2a.tile([P, CT * SRC], bf16, tag="enct2")
                                nc.sync.dma_start(ca_enct[:], d_enct[:])
                                v_proj(ca_enct, SRC, ca_wv, ca_v, KT_CA)

                            with tc.tile_pool(name="caW2b", bufs=1) as caW2b:
                                ca_wq = caW2b.tile([P, CT * D_MODEL], bf16, tag="wq")
                                nc.sync.dma_start(ca_wq[:], dw["ca_wq"][:])
                                proj_T(ca_wq, o1_bf, NQ, ca_qt, NQ, 1)

                            attention(ca_qt, ca_kt, ca_v, KT_CA, ctxn, probs_pool,
                                      rec_pool, mask_ca)

                            with tc.tile_pool(name="caQ", bufs=1) as caQ:
                                wo = caQ.tile([P, CT * D_MODEL], bf16, tag="wo")
                                nc.sync.dma_start(wo[:], dw["ca_wo"][:])
                                res2 = caQ.tile([P, CT * NQ], f32, tag="res2")
                                for f in range(CT):
                                    ps = ps_proj.tile([P, NQ], f32, tag="proj")
                                    for c in range(CT):
                                        nc.tensor.matmul(
                                            ps[:, :],
                                            wo[:, c * D_MODEL + f * P: c * D_MODEL + (f + 1) * P],
                                            ctxn[:, c * NQ:(c + 1) * NQ],
                                            start=(c == 0), stop=(c == CT - 1))
                                    nc.vector.tensor_tensor(
                                        res2[:, f * NQ:(f + 1) * NQ], ps[:, :],
                                        o1_32[:, f * NQ:(f + 1) * NQ], op=OP.add)
                                layer_norm(res2, o2_32, o2_bf, caQ, 1)

                # ---------------- FFN ----------------
                with tc.tile_pool(name="hid_pool", bufs=1) as hid_pool:
                    hid = hid_pool.tile([P, FFT * NQ], bf16, tag="hid")
                    with tc.tile_pool(name="f1", bufs=1) as f1:
                        w1 = f1.tile([P, CT * D_FF], bf16, tag="w1")
                        nc.sync.dma_start(w1[:], dw["ffn_w1"][:])
                        for ff in range(FFT):
                            ps = ps_proj.tile([P, NQ], f32, tag="proj")
                            for c in range(CT):
                                nc.tensor.matmul(
                                    ps[:, :],
                                    w1[:, c * D_FF + ff * P: c * D_FF + (ff + 1) * P],
                                    o2_bf[:, c * NQ:(c + 1) * NQ],
                                    start=(c == 0), stop=(c == CT - 1))
                            nc.scalar.activation(
                                hid[:, ff * NQ:(ff + 1) * NQ], ps[:, :], AF.Relu,
                                bias=fbias[:, ff:ff + 1])
                    with tc.tile_pool(name="f2", bufs=1) as f2:
                        w2 = f2.tile([P, FFT * D_MODEL], bf16, tag="w2")
                        nc.sync.dma_start(w2[:], dw["ffn_w2"][:])
                        res3 = f2.tile([P, CT * NQ], f32, tag="res3")
                        o3_32 = f2.tile([P, CT * NQ], f32, tag="o3_32")
                        for f in range(CT):
                            ps = ps_proj.tile([P, NQ], f32, tag="proj")
                            for c in range(FFT):
                                nc.tensor.matmul(
                                    ps[:, :],
                                    w2[:, c * D_MODEL + f * P: c * D_MODEL + (f + 1) * P],
                                    hid[:, c * NQ:(c + 1) * NQ],
                                    start=(c == 0), stop=(c == FFT - 1))
                            nc.vector.scalar_tensor_tensor(
                                res3[:, f * NQ:(f + 1) * NQ], ps[:, :],
                                fbias[:, FFT + f:FFT + f + 1],
                                o2_32[:, f * NQ:(f + 1) * NQ],
                                op0=OP.add, op1=OP.add)
                        layer_norm(res3, o3_32, None, f2, 2)
                        nc.sync.dma_start(d_out[:], o3_32[:])

    _split_multi_waits(nc, mybir)
    return nc


def _ln_is_trivial(g, b):
    return bool(np.all(np.asarray(g) == 1.0) and np.all(np.asarray(b) == 0.0))


def _to_blocks(a, width):
    """[n_tiles*128, width] -> [128, n_tiles*width] column-block layout."""
    n = a.shape[0] // P
    return np.ascontiguousarray(
        a.reshape(n, P, width).transpose(1, 0, 2).reshape(P, n * width))


def kernel(**inputs):
    from concourse import bass_utils

    x = np.asarray(inputs["dec_layer_inputs"], np.float32)       # [B, TGT, DM]
    enc = np.asarray(inputs["enc_outputs"], np.float32)          # [B, SRC, DM]
    m_sa = np.asarray(inputs["dec_self_attn_mask"], np.float32)  # [B,1,TGT,TGT]
    m_ca = np.asarray(inputs["dec_enc_attn_mask"], np.float32)   # [B,1,TGT,SRC]

    with_mask_sa = bool(np.any(m_sa))
    with_mask_ca = bool(np.any(m_ca))
    with_ln_affine = not (
        _ln_is_trivial(inputs["ln1_g"], inputs["ln1_b"])
        and _ln_is_trivial(inputs["ln2_g"], inputs["ln2_b"])
        and _ln_is_trivial(inputs["ln3_g"], inputs["ln3_b"]))

    key = (with_mask_sa, with_mask_ca, with_ln_affine)
    if key not in _BUILD_CACHE:
        _BUILD_CACHE[key] = _build(*key)
    nc = _BUILD_CACHE[key]

    bf = ml_dtypes.bfloat16

    def wblocks(name, width):
        return _to_blocks(np.asarray(inputs[name], np.float32), width).astype(bf)

    shared = {
        "sa_wq": wblocks("sa_wq", D_MODEL), "sa_wk": wblocks("sa_wk", D_MODEL),
        "sa_wv": wblocks("sa_wv", D_MODEL), "sa_wo": wblocks("sa_wo", D_MODEL),
        "ca_wq": wblocks("ca_wq", D_MODEL), "ca_wk": wblocks("ca_wk", D_MODEL),
        "ca_wv": wblocks("ca_wv", D_MODEL), "ca_wo": wblocks("ca_wo", D_MODEL),
        "ffn_w1": wblocks("ffn_w1", D_FF),
        "ffn_w2": wblocks("ffn_w2", D_MODEL),
        "ones32": np.ones((P, P), np.float32),
        "onesbf": np.ones((P, 8), bf),
    }
    fb = np.zeros((P, FFT + CT + 1), np.float32)
    fb[:, FFT + CT] = EPS
    fb[:, :FFT] = np.asarray(inputs["ffn_b1"], np.float32).reshape(FFT, P).T
    fb[:, FFT:FFT + CT] = np.asarray(inputs["ffn_b2"], np.float32).reshape(CT, P).T
    shared["fbias"] = fb
    if with_ln_affine:
        lp = np.zeros((P, 6 * CT), np.float32)
        for i, nm in enumerate(("ln1_g", "ln1_b", "ln2_g", "ln2_b", "ln3_g", "ln3_b")):
            lp[:, i * CT:(i + 1) * CT] = np.asarray(inputs[nm], np.float32).reshape(CT, P).T
        shared["lnp"] = lp

    in_maps = []
    for core in range(8):
        b, half = divmod(core, 2)
        q0 = half * NQ
        xT = x[b].T                      # [DM, TGT]
        encT = enc[b].T                  # [DM, SRC]
        im = dict(shared)
        im["xq"] = _to_blocks(np.ascontiguousarray(xT[:, q0:q0 + NQ]), NQ).astype(bf)
        im["xq32"] = _to_blocks(np.ascontiguousarray(xT[:, q0:q0 + NQ]), NQ)
        im["xt"] = _to_blocks(np.ascontiguousarray(xT), TGT).astype(bf)
        im["enct"] = _to_blocks(np.ascontiguousarray(encT), SRC).astype(bf)
        if with_mask_sa:
            im["mask_sa"] = _to_blocks(np.ascontiguousarray(m_sa[b, 0].T[:, q0:q0 + NQ]), NQ)
        if with_mask_ca:
            im["mask_ca"] = _to_blocks(np.ascontiguousarray(m_ca[b, 0].T[:, q0:q0 + NQ]), NQ)
        in_maps.append(im)

    trace = bool(int(os.environ.get("KERNEL_TRACE", "0")))
    res = bass_utils.run_bass_kernel_spmd(
        nc, in_maps, core_ids=list(range(8)), trace=trace)
    kernel.last_results = res

    out = np.empty((B, TGT, D_MODEL), np.float32)
    for core in range(8):
        b, half = divmod(core, 2)
        q0 = half * NQ
        o = np.asarray(res.results[core]["out"])   # [128, CT*NQ]
        oT = o.reshape(P, CT, NQ).transpose(1, 0, 2).reshape(D_MODEL, NQ)
        out[b, q0:q0 + NQ, :] = oT.T
    return out


# revision 16
# speedup vs baseline: 1.3799x; 1.0844x over previous
"""Trainium2 Bass kernel for a transformer decoder layer (self-attn +
cross-attn + FFN, post-LN), SPMD across 8 NeuronCores.

Sharding: core = (batch b, query-half): each core handles 512 of the 1024
target positions of one batch element, with the full key/value context.
No collectives; the gather is done on host.

On-device layout: activations are kept TRANSPOSED [features(part) x
tokens(free)] so every linear is out^T[f,:] += W[c,f-block]^T @ act^T[c,:]
with the stored [in,out] weight as the stationary operand.  Attention scores
are computed as S^T [keys(part) x queries(free)]; softmax denominators come
from a ones-column matmul; per-query normalization / LayerNorm row
broadcasts use K=1 matmuls.
"""

import os
import sys

for _p in ("/opt/trn_rl_repo", "/root/.axon_site/_ro/trn_rl_repo"):
    if os.path.isdir(_p) and _p not in sys.path:
        sys.path.insert(0, _p)

import numpy as np
import ml_dtypes

D_MODEL = 1024
N_HEADS = 16
DH = 64
D_FF = 4096
B = 4
TGT = 1024
SRC = 2048
EPS = 1e-5

P = 128
NQ = TGT // 2          # queries per core = 512
CT = D_MODEL // P      # 8 feature tiles
KT_SA = TGT // P       # 8 key tiles (self)
KT_CA = SRC // P       # 16 key tiles (cross)
FFT = D_FF // P        # 32 ffn tiles

_BUILD_CACHE = {}


def _patch_tile_drain(tile):
    """walrus (CoreV3) rejects multiple sync-waits on one TPB_CTRL Drain;
    split the Tile tail drain into one drain per semaphore wait."""
    if getattr(tile.TileContext, "_drain_patch_installed", False):
        return

    def _drain_and_barrier(self, tick_clock, wait_clock):
        drain_inst = self.nc.sync.drain()
        wait_clock.add_sem_waits(
            drain_inst.ins, tile.ScopedClock({None: tick_clock.global_clock})
        )
        si = drain_inst.ins.sync_info
        if si is not None and si.on_wait and len(si.on_wait) > 1:
            waits = list(si.on_wait)
            del si.on_wait[1:]
            for w in waits[1:]:
                extra = self.nc.sync.drain()
                if extra.ins.sync_info is None:
                    import concourse.mybir as mybir
                    extra.ins.sync_info = mybir.SyncInfo(on_wait=[], on_update=[])
                extra.ins.sync_info.on_wait.append(w)

        self.nc.all_engine_barrier()
        assert self.sems is not None
        popped = self.nc._tile_sem_poison_stack.pop()
        assert popped is self._sem_poison
        self.nc.clear_and_free_semaphores(list(self.sems.allocated().values()))
        self.nc.all_engine_barrier()

    tile.TileContext._drain_and_barrier = _drain_and_barrier
    tile.TileContext._drain_patch_installed = True


def _split_multi_waits(nc, mybir):
    """This walrus build accepts a single sync-wait per instruction; hoist
    extra waits onto standalone EventSemaphore carriers inserted just before
    the instruction on the same engine."""
    n = 0
    for f in nc.m.functions:
        for blk in f.blocks:
            out = []
            for ins in blk.instructions:
                si = getattr(ins, "sync_info", None)
                if si is not None and si.on_wait and len(si.on_wait) > 1:
                    waits = list(si.on_wait)
                    del si.on_wait[:-1]
                    for w in waits[:-1]:
                        n += 1
                        carrier = mybir.InstEventSemaphore(
                            name=f"I-waitsplit-{n}",
                            engine=ins.engine,
                            ins=[],
                            outs=[],
                            sync_info=mybir.SyncInfo(on_wait=[w], on_update=[]),
                        )
                        out.append(carrier)
                out.append(ins)
            blk.instructions = out
    return n


def _build(with_mask_sa, with_mask_ca, with_ln_affine):
    import concourse.bass as bass
    import concourse.mybir as mybir
    import concourse.tile as tile

    _patch_tile_drain(tile)

    f32 = mybir.dt.float32
    bf16 = mybir.dt.bfloat16
    AF = mybir.ActivationFunctionType
    OP = mybir.AluOpType

    nc = bass.Bass("TRN2", target_bir_lowering=False, debug=False, num_devices=1)

    # ---- DRAM I/O (column-block layout: [128, n_tiles*width]) ----
    d_xq32 = nc.dram_tensor("xq32", [P, CT * NQ], f32, kind="ExternalInput").ap()
    d_xt = nc.dram_tensor("xt", [P, CT * TGT], bf16, kind="ExternalInput").ap()
    d_enct = nc.dram_tensor("enct", [P, CT * SRC], bf16, kind="ExternalInput").ap()
    dw = {}
    for name in ("sa_wq", "sa_wk", "sa_wv", "sa_wo", "ca_wq", "ca_wk", "ca_wv", "ca_wo"):
        dw[name] = nc.dram_tensor(name, [P, CT * D_MODEL], bf16, kind="ExternalInput").ap()
    dw["ffn_w1"] = nc.dram_tensor("ffn_w1", [P, CT * D_FF], bf16, kind="ExternalInput").ap()
    dw["ffn_w2"] = nc.dram_tensor("ffn_w2", [P, FFT * D_MODEL], bf16, kind="ExternalInput").ap()
    d_fb = nc.dram_tensor("fbias", [P, FFT + CT + 1], f32, kind="ExternalInput").ap()
    d_ones32 = nc.dram_tensor("ones32", [P, P], f32, kind="ExternalInput").ap()
    d_onesbf = nc.dram_tensor("onesbf", [P, 8], bf16, kind="ExternalInput").ap()
    d_mask_sa = d_mask_ca = None
    if with_mask_sa:
        d_mask_sa = nc.dram_tensor("mask_sa", [P, KT_SA * NQ], f32, kind="ExternalInput").ap()
    if with_mask_ca:
        d_mask_ca = nc.dram_tensor("mask_ca", [P, KT_CA * NQ], f32, kind="ExternalInput").ap()
    d_lnp = None
    if with_ln_affine:
        d_lnp = nc.dram_tensor("lnp", [P, 6 * CT], f32, kind="ExternalInput").ap()
    d_out = nc.dram_tensor("out", [P, CT * NQ], f32, kind="ExternalOutput").ap()

    with tile.TileContext(nc) as tc:
        with (
            tc.tile_pool(name="const", bufs=1) as cpool,
            tc.tile_pool(name="ps_proj", bufs=2, space="PSUM") as ps_proj,
            tc.tile_pool(name="ps_sc", bufs=2, space="PSUM") as ps_sc,
            tc.tile_pool(name="ps_ctx", bufs=1, space="PSUM") as ps_ctx,
            tc.tile_pool(name="ps_aux", bufs=1, space="PSUM") as ps_aux,
        ):
            ones32 = cpool.tile([P, P], f32, tag="ones32")
            nc.sync.dma_start(ones32[:], d_ones32[:])
            onesbf = cpool.tile([P, 8], bf16, tag="onesbf")
            nc.sync.dma_start(onesbf[:], d_onesbf[:])
            fbias = cpool.tile([P, FFT + CT + 1], f32, tag="fbias")
            nc.sync.dma_start(fbias[:], d_fb[:])
            lnp = None
            if with_ln_affine:
                lnp = cpool.tile([P, 6 * CT], f32, tag="lnp")
                nc.sync.dma_start(lnp[:], d_lnp[:])


            def act_recip(out, in_):
                eng = nc.scalar
                ins_ = [eng.lower_ap(in_)]
                for argv in (0.0, 1.0, 0.0):
                    ins_.append(mybir.ImmediateValue(dtype=mybir.dt.float32, value=argv))
                return eng.add_instruction(mybir.InstActivation(
                    name=nc.get_next_instruction_name(),
                    func=AF.Reciprocal, ins=ins_, outs=[eng.lower_ap(out)]))

            def layer_norm(res_sb, out32_sb, outbf_sb, tmp_pool, ln_idx):
                """res_sb [128, CT*NQ] f32 -> out32 (f32) and optional outbf
                (bf16); LayerNorm over the feature (partition x tile) axis."""
                mean_ps = ps_ctx.tile([P, NQ], f32, tag="ctx")
                sq_ps = ps_aux.tile([P, NQ], f32, tag="aux")
                for c in range(CT):
                    rsl = res_sb[:, c * NQ:(c + 1) * NQ]
                    sq = tmp_pool.tile([P, NQ], f32, tag="sq", bufs=2)
                    nc.scalar.activation(sq[:], rsl, AF.Square)
                    nc.tensor.matmul(mean_ps[0:1, :], ones32[:, 0:1], rsl,
                                     start=(c == 0), stop=(c == CT - 1))
                    nc.tensor.matmul(sq_ps[0:1, :], ones32[:, 0:1], sq[:],
                                     start=(c == 0), stop=(c == CT - 1))
                rows = tmp_pool.tile([1, 4 * NQ], f32, tag="rows")
                m = rows[:, 0:NQ]
                s = rows[:, NQ:2 * NQ]
                a = rows[:, 2 * NQ:3 * NQ]
                b = rows[:, 3 * NQ:4 * NQ]
                nc.vector.tensor_scalar_mul(m, mean_ps[0:1, :], 1.0 / D_MODEL)
                nc.vector.tensor_scalar_mul(s, sq_ps[0:1, :], 1.0 / D_MODEL)
                nc.vector.tensor_tensor(a, m, m, op=OP.mult)        # mean^2
                nc.vector.tensor_tensor(a, s, a, op=OP.subtract)    # var
                nc.scalar.activation(a, a, AF.Sqrt, bias=fbias[0:1, FFT + CT:FFT + CT + 1])  # sqrt(var+eps)
                act_recip(a, a)                                     # rstd
                nc.vector.tensor_tensor(b, m, a, op=OP.mult)
                nc.vector.tensor_scalar_mul(b, b, -1.0)             # -mean*rstd
                bca_ps = ps_aux.tile([P, NQ], f32, tag="aux")
                bcb_ps = ps_ctx.tile([P, NQ], f32, tag="ctx")
                nc.tensor.matmul(bca_ps[:, :], ones32[0:1, :], a, start=True, stop=True)
                nc.tensor.matmul(bcb_ps[:, :], ones32[0:1, :], b, start=True, stop=True)
                bca = tmp_pool.tile([P, NQ], f32, tag="bca")
                bcb = tmp_pool.tile([P, NQ], f32, tag="bcb")
                nc.scalar.copy(bca[:], bca_ps[:, :])
                nc.scalar.copy(bcb[:], bcb_ps[:, :])
                for c in range(CT):
                    rsl = res_sb[:, c * NQ:(c + 1) * NQ]
                    o32 = out32_sb[:, c * NQ:(c + 1) * NQ]
                    t = tmp_pool.tile([P, NQ], f32, tag="sq", bufs=2)
                    nc.vector.tensor_tensor(t[:], rsl, bca[:], op=OP.mult)
                    nc.vector.tensor_tensor(o32, t[:], bcb[:], op=OP.add)
                    if with_ln_affine:
                        g = lnp[:, (2 * ln_idx) * CT + c:(2 * ln_idx) * CT + c + 1]
                        be = lnp[:, (2 * ln_idx + 1) * CT + c:(2 * ln_idx + 1) * CT + c + 1]
                        nc.vector.tensor_scalar(o32, o32, g, be, op0=OP.mult, op1=OP.add)
                    if outbf_sb is not None:
                        nc.scalar.copy(outbf_sb[:, c * NQ:(c + 1) * NQ], o32)

            def attention(qt, kt, v, nkt, ctxn, probs_pool, rec_pool, mask_sb,
                          filler=None):
                """qt [128, CT*NQ] bf16, kt [128, CT*(nkt*128)] bf16,
                v [128, nkt*D_MODEL] bf16 -> ctxn [128, CT*NQ] bf16.

                Software-pipelined: scores+exp for step k are issued before the
                ctx/den matmuls of step k-1, so the PE never stalls on the exp.
                `filler` is an optional generator; one next() per pair lets
                independent PE work (e.g. the next phase's projections) fill
                the exp-bound gaps."""
                KW = nkt * P  # key width per feature block of kt

                def emit_cd(k, p2):
                    for half in range(2):
                        h_off = (2 * pair + half) * DH
                        psl = slice(64 * half, 64 * half + 64)
                        pq = p2[:, half * NQ:(half + 1) * NQ]
                        nc.tensor.matmul(
                            ctx_ps[psl, :],
                            v[:, k * D_MODEL + h_off: k * D_MODEL + h_off + DH],
                            pq, start=(k == 0), stop=(k == nkt - 1))
                        nc.tensor.matmul(
                            den_ps[64 * half:64 * half + 1, :],
                            onesbf[:, 0:1],
                            pq, start=(k == 0), stop=(k == nkt - 1))

                for pair in range(N_HEADS // 2):
                    t = pair
                    ctx_ps = ps_ctx.tile([P, NQ], f32, tag="ctx")
                    den_ps = ps_aux.tile([P, NQ], f32, tag="aux")
                    prev = None
                    for k in range(nkt):
                        s2_ps = ps_sc.tile([P, 2 * NQ], f32, tag="sc")
                        for half in range(2):
                            psl = slice(64 * half, 64 * half + 64)
                            nc.tensor.matmul(
                                s2_ps[:, half * NQ:(half + 1) * NQ],
                                kt[psl, t * KW + k * P: t * KW + (k + 1) * P],
                                qt[psl, t * NQ:(t + 1) * NQ],
                                start=True, stop=True)
                        if mask_sb is not None:
                            for half in range(2):
                                nc.vector.tensor_tensor(
                                    s2_ps[:, half * NQ:(half + 1) * NQ],
                                    s2_ps[:, half * NQ:(half + 1) * NQ],
                                    mask_sb[:, k * NQ:(k + 1) * NQ], op=OP.add)
                        p2 = probs_pool.tile([P, 2 * NQ], bf16, tag="p")
                        nc.scalar.activation(p2[:], s2_ps[:, :], AF.Exp,
                                             scale=1.0 / 8.0)
                        if prev is not None:
                            emit_cd(*prev)
                        prev = (k, p2)
                    emit_cd(*prev)
                    # normalize: 1/den broadcast over the pair's 2x64 rows
                    rec = rec_pool.tile([P, NQ], f32, tag="rec")
                    act_recip(rec[:, :], den_ps[:, :])
                    bc_ps = ps_sc.tile([P, NQ], f32, tag="sc")
                    nc.tensor.matmul(bc_ps[0:64, :], ones32[0:1, 0:64], rec[0:1, :],
                                     start=True, stop=True)
                    nc.tensor.matmul(bc_ps[64:128, :], ones32[64:65, 0:64], rec[64:65, :],
                                     start=True, stop=True)
                    bc = rec_pool.tile([P, NQ], f32, tag="bc")
                    nc.scalar.copy(bc[:], bc_ps[:, :])
                    nc.vector.tensor_tensor(ctxn[:, t * NQ:(t + 1) * NQ],
                                            ctx_ps[:, :], bc[:], op=OP.mult)
                    if filler is not None:
                        next(filler, None)

            def proj_T(w_sb, act_sb, actw, dst_sb, dstw, nchunks):
                """dst^T[f-block, chunk] = sum_c W[c, f]^T @ act^T[c, chunk]"""
                for f in range(CT):
                    for n in range(nchunks):
                        ps = ps_proj.tile([P, NQ], f32, tag="proj")
                        for c in range(CT):
                            nc.tensor.matmul(
                                ps[:, :],
                                w_sb[:, c * D_MODEL + f * P: c * D_MODEL + (f + 1) * P],
                                act_sb[:, c * actw + n * NQ: c * actw + (n + 1) * NQ],
                                start=(c == 0), stop=(c == CT - 1))
                        nc.vector.tensor_copy(
                            dst_sb[:, f * dstw + n * NQ: f * dstw + (n + 1) * NQ],
                            ps[:, :])

            def v_proj(act_sb, actw, wv_sb, v_sb, nkt):
                """V[k-block, :] = act[k,:] @ Wv  (normal layout)."""
                for k in range(nkt):
                    for n in range(D_MODEL // NQ):
                        ps = ps_proj.tile([P, NQ], f32, tag="proj")
                        for c in range(CT):
                            nc.tensor.matmul(
                                ps[:, :],
                                act_sb[:, c * actw + k * P: c * actw + (k + 1) * P],
                                wv_sb[:, c * D_MODEL + n * NQ: c * D_MODEL + (n + 1) * NQ],
                                start=(c == 0), stop=(c == CT - 1))
                        nc.vector.tensor_copy(
                            v_sb[:, k * D_MODEL + n * NQ: k * D_MODEL + (n + 1) * NQ],
                            ps[:, :])

            with tc.tile_pool(name="s2out", bufs=1) as s2out:
                o2_32 = s2out.tile([P, CT * NQ], f32, tag="o2_32")
                o2_bf = s2out.tile([P, CT * NQ], bf16, tag="o2_bf")

                with (
                    tc.tile_pool(name="ctxn_pool", bufs=1) as ctxn_pool,
                    tc.tile_pool(name="probs", bufs=3) as probs_pool,
                    tc.tile_pool(name="rec", bufs=1) as rec_pool,
                    tc.tile_pool(name="ca_ktP", bufs=1) as ca_ktP,
                ):
                    ctxn = ctxn_pool.tile([P, CT * NQ], bf16, tag="ctxn")
                    ca_kt = ca_ktP.tile([P, CT * SRC], bf16, tag="kt")

                    # ---------------- self attention ----------------
                    with tc.tile_pool(name="saQpre", bufs=1) as saQpre:
                      sa_wo = saQpre.tile([P, CT * D_MODEL], bf16, tag="wo")
                      nc.sync.dma_start(sa_wo[:], dw["sa_wo"][:])
                      with tc.tile_pool(name="saAtt", bufs=1) as saAtt:
                        qt = saAtt.tile([P, CT * NQ], bf16, tag="qt")
                        kt = saAtt.tile([P, CT * TGT], bf16, tag="kt")
                        v = saAtt.tile([P, KT_SA * D_MODEL], bf16, tag="v")
                        mask_sa = None
                        if with_mask_sa:
                            mask_sa = saAtt.tile([P, KT_SA * NQ], f32, tag="msa")
                            nc.sync.dma_start(mask_sa[:], d_mask_sa[:])

                        with tc.tile_pool(name="saIn", bufs=1) as saIn:
                            xt = saIn.tile([P, CT * TGT], bf16, tag="xt")
                            nc.sync.dma_start(xt[:], d_xt[:])
                            # queries are the first NQ tokens of (rolled) xt
                            wq = saIn.tile([P, CT * D_MODEL], bf16, tag="wq")
                            nc.sync.dma_start(wq[:], dw["sa_wq"][:])
                            wk = saIn.tile([P, CT * D_MODEL], bf16, tag="wk")
                            nc.sync.dma_start(wk[:], dw["sa_wk"][:])
                            wv = saIn.tile([P, CT * D_MODEL], bf16, tag="wv")
                            nc.sync.dma_start(wv[:], dw["sa_wv"][:])
                            proj_T(wq, xt, TGT, qt, NQ, 1)
                            proj_T(wk, xt, TGT, kt, TGT, TGT // NQ)
                            v_proj(xt, TGT, wv, v, KT_SA)

                        with tc.tile_pool(name="caKV", bufs=1) as caKV:
                            enct = caKV.tile([P, CT * SRC], bf16, tag="enct")
                            nc.sync.dma_start(enct[:], d_enct[:])
                            ca_wk = caKV.tile([P, CT * D_MODEL], bf16, tag="wk")
                            nc.sync.dma_start(ca_wk[:], dw["ca_wk"][:])

                            def ca_kt_gen():
                                """cross-attn K^T chains, 4 per next()."""
                                i = 0
                                for f in range(CT):
                                    for n in range(SRC // NQ):
                                        ps = ps_proj.tile([P, NQ], f32, tag="proj")
                                        for c in range(CT):
                                            nc.tensor.matmul(
                                                ps[:, :],
                                                ca_wk[:, c * D_MODEL + f * P: c * D_MODEL + (f + 1) * P],
                                                enct[:, c * SRC + n * NQ: c * SRC + (n + 1) * NQ],
                                                start=(c == 0), stop=(c == CT - 1))
                                        nc.vector.tensor_copy(
                                            ca_kt[:, f * SRC + n * NQ: f * SRC + (n + 1) * NQ],
                                            ps[:, :])
                                        i += 1
                                        if i % 4 == 0:
                                            yield

                            gen = ca_kt_gen()
                            attention(qt, kt, v, KT_SA, ctxn, probs_pool,
                                      rec_pool, mask_sa, filler=gen)
                            for _ in gen:  # finish any remaining chains
                                pass

                    with tc.tile_pool(name="s1out", bufs=1) as s1out:
                        o1_32 = s1out.tile([P, CT * NQ], f32, tag="o1_32")
                        o1_bf = s1out.tile([P, CT * NQ], bf16, tag="o1_bf")

                        with tc.tile_pool(name="saQ", bufs=1) as saQ:
                            wo = sa_wo
                            xq32 = saQ.tile([P, CT * NQ], f32, tag="xq32")
                            nc.sync.dma_start(xq32[:], d_xq32[:])
                            res1 = saQ.tile([P, CT * NQ], f32, tag="res1")
                            for f in range(CT):
                                ps = ps_proj.tile([P, NQ], f32, tag="proj")
                                for c in range(CT):
                                    nc.tensor.matmul(
                                        ps[:, :],
                                        wo[:, c * D_MODEL + f * P: c * D_MODEL + (f + 1) * P],
                                        ctxn[:, c * NQ:(c + 1) * NQ],
                                        start=(c == 0), stop=(c == CT - 1))
                                nc.vector.tensor_tensor(
                                    res1[:, f * NQ:(f + 1) * NQ], ps[:, :],
                                    xq32[:, f * NQ:(f + 1) * NQ], op=OP.add)
                            layer_norm(res1, o1_32, o1_bf, saQ, 0)

                        # ---------------- cross attention ----------------
                        with tc.tile_pool(name="caVQ", bufs=1) as caVQ:
                            ca_qt = caVQ.tile([P, CT * NQ], bf16, tag="qt")
                            ca_v = caVQ.tile([P, KT_CA * D_MODEL], bf16, tag="v")
                            mask_ca = None
                            if with_mask_ca:
                                mask_ca = caVQ.tile([P, KT_CA * NQ], f32, tag="mca")
                                nc.sync.dma_start(mask_ca[:], d_mask_ca[:])

                            with tc.tile_pool(name="caW2a", bufs=1) as caW2a:
                                ca_wv = caW2a.tile([P, CT * D_MODEL], bf16, tag="wv")
                                nc.sync.dma_start(ca_wv[:], dw["ca_wv"][:])
                                ca_enct = caW2a.tile([P, CT * SRC], bf16, tag="enct2")
                                nc.sync.dma_start(ca_enct[:], d_enct[:])
                                v_proj(ca_enct, SRC, ca_wv, ca_v, KT_CA)

                            with tc.tile_pool(name="caW2b", bufs=1) as caW2b:
                                ca_wq = caW2b.tile([P, CT * D_MODEL], bf16, tag="wq")
                                nc.sync.dma_start(ca_wq[:], dw["ca_wq"][:])
                                proj_T(ca_wq, o1_bf, NQ, ca_qt, NQ, 1)

                            with tc.tile_pool(name="caQpre", bufs=1) as caQpre:
                              wo = caQpre.tile([P, CT * D_MODEL], bf16, tag="wo")
                              nc.sync.dma_start(wo[:], dw["ca_wo"][:])
                              attention(ca_qt, ca_kt, ca_v, KT_CA, ctxn, probs_pool,
                                        rec_pool, mask_ca)

                              with tc.tile_pool(name="caQ", bufs=1) as caQ:
                                res2 = caQ.tile([P, CT * NQ], f32, tag="res2")
                                for f in range(CT):
                                    ps = ps_proj.tile([P, NQ], f32, tag="proj")
                                    for c in range(CT):
                                        nc.tensor.matmul(
                                            ps[:, :],
                                            wo[:, c * D_MODEL + f * P: c * D_MODEL + (f + 1) * P],
                                            ctxn[:, c * NQ:(c + 1) * NQ],
                                            start=(c == 0), stop=(c == CT - 1))
                                    nc.vector.tensor_tensor(
                                        res2[:, f * NQ:(f + 1) * NQ], ps[:, :],
                                        o1_32[:, f * NQ:(f + 1) * NQ], op=OP.add)
                                layer_norm(res2, o2_32, o2_bf, caQ, 1)


                # ---------------- FFN ----------------
                with tc.tile_pool(name="hid_pool", bufs=1) as hid_pool:
                    hid = hid_pool.tile([P, FFT * NQ], bf16, tag="hid")
                    with tc.tile_pool(name="f2w", bufs=1) as f2w:
                      w2 = f2w.tile([P, FFT * D_MODEL], bf16, tag="w2")
                      nc.sync.dma_start(w2[:], dw["ffn_w2"][:])
                      with tc.tile_pool(name="f1", bufs=1) as f1:
                        w1 = f1.tile([P, CT * D_FF], bf16, tag="w1")
                        # chunked ff-group-major so early hid chains start
                        # before the whole 8 MB weight lands
                        for ffg in range(CT):
                            for c in range(CT):
                                nc.sync.dma_start(
                                    w1[:, c * D_FF + ffg * NQ: c * D_FF + (ffg + 1) * NQ],
                                    dw["ffn_w1"][:, c * D_FF + ffg * NQ: c * D_FF + (ffg + 1) * NQ])
                        for ff in range(FFT):
                            ps = ps_proj.tile([P, NQ], f32, tag="proj")
                            for c in range(CT):
                                nc.tensor.matmul(
                                    ps[:, :],
                                    w1[:, c * D_FF + ff * P: c * D_FF + (ff + 1) * P],
                                    o2_bf[:, c * NQ:(c + 1) * NQ],
                                    start=(c == 0), stop=(c == CT - 1))
                            nc.scalar.activation(
                                hid[:, ff * NQ:(ff + 1) * NQ], ps[:, :], AF.Relu,
                                bias=fbias[:, ff:ff + 1])
                      with tc.tile_pool(name="f2", bufs=1) as f2:
                        res3 = f2.tile([P, CT * NQ], f32, tag="res3")
                        o3_32 = f2.tile([P, CT * NQ], f32, tag="o3_32")
                        for f in range(CT):
                            ps = ps_proj.tile([P, NQ], f32, tag="proj")
                            for c in range(FFT):
                                nc.tensor.matmul(
                                    ps[:, :],
                                    w2[:, c * D_MODEL + f * P: c * D_MODEL + (f + 1) * P],
                                    hid[:, c * NQ:(c + 1) * NQ],
                                    start=(c == 0), stop=(c == FFT - 1))
                            nc.vector.scalar_tensor_tensor(
                                res3[:, f * NQ:(f + 1) * NQ], ps[:, :],
                                fbias[:, FFT + f:FFT + f + 1],
                                o2_32[:, f * NQ:(f + 1) * NQ],
                                op0=OP.add, op1=OP.add)
                        layer_norm(res3, o3_32, None, f2, 2)
                        nc.sync.dma_start(d_out[:], o3_32[:])

    _split_multi_waits(nc, mybir)
    return nc


def _ln_is_trivial(g, b):
    return bool(np.all(np.asarray(g) == 1.0) and np.all(np.asarray(b) == 0.0))


def _to_blocks(a, width):
    """[n_tiles*128, width] -> [128, n_tiles*width] column-block layout."""
    n = a.shape[0] // P
    return np.ascontiguousarray(
        a.reshape(n, P, width).transpose(1, 0, 2).reshape(P, n * width))


def kernel(**inputs):
    from concourse import bass_utils

    x = np.asarray(inputs["dec_layer_inputs"], np.float32)       # [B, TGT, DM]
    enc = np.asarray(inputs["enc_outputs"], np.float32)          # [B, SRC, DM]
    m_sa = np.asarray(inputs["dec_self_attn_mask"], np.float32)  # [B,1,TGT,TGT]
    m_ca = np.asarray(inputs["dec_enc_attn_mask"], np.float32)   # [B,1,TGT,SRC]

    with_mask_sa = bool(np.any(m_sa))
    with_mask_ca = bool(np.any(m_ca))
    with_ln_affine = not (
        _ln_is_trivial(inputs["ln1_g"], inputs["ln1_b"])
        and _ln_is_trivial(inputs["ln2_g"], inputs["ln2_b"])
        and _ln_is_trivial(inputs["ln3_g"], inputs["ln3_b"]))

    key = (with_mask_sa, with_mask_ca, with_ln_affine)
    if key not in _BUILD_CACHE:
        _BUILD_CACHE[key] = _build(*key)
    nc = _BUILD_CACHE[key]

    bf = ml_dtypes.bfloat16

    def wblocks(name, width):
        return _to_blocks(np.asarray(inputs[name], np.float32), width).astype(bf)

    shared = {
        "sa_wq": wblocks("sa_wq", D_MODEL), "sa_wk": wblocks("sa_wk", D_MODEL),
        "sa_wv": wblocks("sa_wv", D_MODEL), "sa_wo": wblocks("sa_wo", D_MODEL),
        "ca_wq": wblocks("ca_wq", D_MODEL), "ca_wk": wblocks("ca_wk", D_MODEL),
        "ca_wv": wblocks("ca_wv", D_MODEL), "ca_wo": wblocks("ca_wo", D_MODEL),
        "ffn_w1": wblocks("ffn_w1", D_FF),
        "ffn_w2": wblocks("ffn_w2", D_MODEL),
        "ones32": np.ones((P, P), np.float32),
        "onesbf": np.ones((P, 8), bf),
    }
    fb = np.zeros((P, FFT + CT + 1), np.float32)
    fb[:, FFT + CT] = EPS
    fb[:, :FFT] = np.asarray(inputs["ffn_b1"], np.float32).reshape(FFT, P).T
    fb[:, FFT:FFT + CT] = np.asarray(inputs["ffn_b2"], np.float32).reshape(CT, P).T
    shared["fbias"] = fb
    if with_ln_affine:
        lp = np.zeros((P, 6 * CT), np.float32)
        for i, nm in enumerate(("ln1_g", "ln1_b", "ln2_g", "ln2_b", "ln3_g", "ln3_b")):
            lp[:, i * CT:(i + 1) * CT] = np.asarray(inputs[nm], np.float32).reshape(CT, P).T
        shared["lnp"] = lp

    in_maps = []
    for core in range(8):
        b, half = divmod(core, 2)
        q0 = half * NQ
        xT = x[b].T                      # [DM, TGT]
        encT = enc[b].T                  # [DM, SRC]
        im = dict(shared)
        xT_roll = np.concatenate([xT[:, q0:], xT[:, :q0]], axis=1)
        im["xq32"] = _to_blocks(np.ascontiguousarray(xT[:, q0:q0 + NQ]), NQ)
        im["xt"] = _to_blocks(np.ascontiguousarray(xT_roll), TGT).astype(bf)
        im["enct"] = _to_blocks(np.ascontiguousarray(encT), SRC).astype(bf)
        if with_mask_sa:
            mT = m_sa[b, 0].T[:, q0:q0 + NQ]
            mT = np.concatenate([mT[q0:], mT[:q0]], axis=0)
            im["mask_sa"] = _to_blocks(np.ascontiguousarray(mT), NQ)
        if with_mask_ca:
            im["mask_ca"] = _to_blocks(np.ascontiguousarray(m_ca[b, 0].T[:, q0:q0 + NQ]), NQ)
        in_maps.append(im)

    trace = bool(int(os.environ.get("KERNEL_TRACE", "0")))
    res = bass_utils.run_bass_kernel_spmd(
        nc, in_maps, core_ids=list(range(8)), trace=trace)
    kernel.last_results = res

    out = np.empty((B, TGT, D_MODEL), np.float32)
    for core in range(8):
        b, half = divmod(core, 2)
        q0 = half * NQ
        o = np.asarray(res.results[core]["out"])   # [128, CT*NQ]
        oT = o.reshape(P, CT, NQ).transpose(1, 0, 2).reshape(D_MODEL, NQ)
        out[b, q0:q0 + NQ, :] = oT.T
    return out
